# revision 1
# baseline (speedup 1.0000x reference)
"""Causal single-head attention (B=4, S=2048, D=1024, fp32 I/O) on 8 trn2 cores.

Sharding: core c = (batch b = c//2, half h = c%2). Each core computes K/V for
the full sequence of its batch and handles 8 query blocks of 128 rows:
blocks {h, h+2, ..., h+14} (even/odd striping balances causal work exactly and
keeps the instruction stream identical across cores — all causal-mask
differences live in a per-core mask input).

Slot j on every core runs the same static schedule: scores over kblocks
[0, 2j+2), softmax, probs^T via PE transpose, attn@V. For h=0 cores the last
kblock of each slot is fully masked (one wasted block per slot); for h=1 it is
the diagonal block.

Host-side prep (not device-timed): x transpose to [D, S], bf16 casts, query
column gather, mask construction, output scatter.
"""

import functools

import ml_dtypes
import numpy as np

import concourse.bass as bass
import concourse.tile as tile
from concourse import bacc, mybir
from concourse.bass_utils import run_bass_kernel_spmd

B, S, D = 4, 2048, 1024
P = 128
NB = S // P          # 16 key blocks per sequence
NSLOT = NB // 2      # 8 query slots per core
NCH = D // P         # 8 contraction chunks of 128
N_CORES = 8
SCALE = 1.0 / 32.0   # 1/sqrt(D)
NEG = -1e9

F32 = mybir.dt.float32
BF16 = mybir.dt.bfloat16
BF = ml_dtypes.bfloat16


def _emit_kernel(nc, tc, xT_d, xq_d, wq_d, wk_d, wv_d, mask_d, ident_d, out_d,
                 cc_bufs=None, cc_split=True, asc=True, ps_rebal=False,
                 cc_local=False):
    # cc_local: TimelineSim-only stand-in — replace each AllGather with DMAs
    # of the same byte volume (local half copied to both replica slots).
    # DRAM views: [(c p) n -> p c n] puts feature chunks on partitions.
    # cc_bufs: (ccin_d, ccout_d) DRAM bounce tensors → KV projected for the
    # local half-sequence only and exchanged pair-wise via AllGather.
    xT = xT_d[:].rearrange("(c p) s -> p c s", p=P)      # [128, 8, S or S/2]
    xq = xq_d[:].rearrange("(c p) q -> p c q", p=P)      # [128, 8, 1024]
    wq = wq_d[:].rearrange("(c p) n -> p c n", p=P)      # [128, 8, 1024]
    wk = wk_d[:].rearrange("(c p) n -> p c n", p=P)
    wv = wv_d[:].rearrange("(c p) n -> p c n", p=P)
    SKV = xT.shape[2]  # sequence cols this core projects K/V for

    singles = tc.alloc_tile_pool(name="singles", bufs=1)
    kv = tc.alloc_tile_pool(name="kv", bufs=1)

    ident_sb = singles.tile([P, P], BF16)
    mask_sb = singles.tile([P, 2 * P], F32)

    # Long-lived activations (bf16): K^T [d,s], Q^T [d,q], V [s,d]
    KT = kv.tile([P, NCH, S], BF16)        # 4 MB
    QT = kv.tile([P, NCH, NSLOT * P], BF16)  # 2 MB
    V = kv.tile([P, NB, D], BF16)          # 4 MB

    cp_eng = [
        lambda o, i: nc.vector.tensor_copy(o, i),
        lambda o, i: nc.scalar.copy(o, i),
    ]

    # ---- Phase A: projections (inputs and weights are bf16 in DRAM) ----
    with tc.tile_pool(name="xw", bufs=1) as xw, tc.tile_pool(name="w", bufs=3) as wpool:
        # Critical-path loads first on BOTH hwdge queues (SP carries weights,
        # Activation carries activations): the ki-outer K loop below starts
        # accumulating as soon as chunk pair 0 lands.
        xT_sb = xw.tile([P, NCH, SKV], BF16)
        wk_sb = wpool.tile([P, NCH, D], BF16, tag="w")
        for ki in range(NCH):
            nc.sync.dma_start(wk_sb[:, ki, :], wk[:, ki, :])
            nc.sync.dma_start(xT_sb[:, ki, :], xT[:, ki, :])
        # everything else streams in behind as coarse one-shot DMAs (HWDGE
        # descriptor time scales with row count, so fewer DMAs = less queue
        # pressure); each lands well before its first use
        nc.sync.dma_start(ident_sb[:], ident_d[:])
        nc.sync.dma_start(mask_sb[:], mask_d[:])
        wv_sb = wpool.tile([P, NCH, D], BF16, tag="w")
        nc.sync.dma_start(wv_sb[:], wv)
        wq_sb = wpool.tile([P, NCH, D], BF16, tag="w")
        nc.sync.dma_start(wq_sb[:], wq)
        xq_sb = xw.tile([P, NCH, NSLOT * P], BF16)  # 2 MB
        nc.sync.dma_start(xq_sb[:], xq)

        if cc_bufs is not None:
            KT_dst = xw.tile([P, NCH, SKV], BF16, tag="kth")
            V_dst = xw.tile([P, SKV // P, D], BF16, tag="vh")
        else:
            KT_dst, V_dst = KT, V

        PAIRS = [[0, 1], [2, 3], [4, 5], [6, 7]]

        def pair_allgather(src_ap, dst_ap):
            # dst layout: [rep][...]; AllGather over the core pair.
            if cc_local:
                for rep in range(2):
                    nc.sync.dma_start(dst_ap[rep], src_ap)
            else:
                nc.gpsimd.collective_compute(
                    "AllGather", mybir.AluOpType.bypass, replica_groups=PAIRS,
                    ins=[src_ap], outs=[dst_ap],
                )

        # All three projections share one ki-outer PSUM pipeline: groups of
        # open accumulations, drains trailing each group.  The first K group
        # is 8 wide so each ki round (~1.7us of PE) outlasts the DMA cadence
        # of a chunk pair (~1.5us) — PE starts after chunk 0 and never waits
        # for the full load.  Later groups are 4 wide so bank rotation keeps
        # pass-to-pass stalls hidden behind the trailing group's drains.
        gcnt = [0]

        def proj_run(ps_proj, pairs, first_group, lhsT_fn, rhs_fn, drain_fn):
            sizes = [first_group] if first_group else []
            left = len(pairs) - first_group
            sizes += [4] * (left // 4)
            idx = 0
            for gs in sizes:
                grp = pairs[idx:idx + gs]
                idx += gs
                tiles = [ps_proj.tile([P, 512], F32, tag="ps",
                                      name=f"pp{gcnt[0]}_{t}")
                         for t in range(len(grp))]
                gcnt[0] += 1
                for ki in range(NCH):
                    for t, pw in enumerate(grp):
                        nc.tensor.matmul(
                            tiles[t][:], lhsT=lhsT_fn(ki, pw),
                            rhs=rhs_fn(ki, pw),
                            start=(ki == 0), stop=(ki == NCH - 1),
                        )
                for t, pw in enumerate(grp):
                    drain_fn(t, pw, tiles[t])

        with tc.tile_pool(name="ps_proj", bufs=8, space="PSUM") as ps_proj:
            # K^T[mi, :] = sum_ki Wk[ki, mi]^T @ xT[ki, :]
            proj_run(
                ps_proj,
                [(mi, w) for mi in range(NCH) for w in range(SKV // 512)],
                8,
                lambda ki, p: wk_sb[:, ki, p[0] * P:(p[0] + 1) * P],
                lambda ki, p: xT_sb[:, ki, p[1] * 512:(p[1] + 1) * 512],
                lambda t, p, tl: cp_eng[t % 2](
                    KT_dst[:, p[0], p[1] * 512:(p[1] + 1) * 512], tl[:]),
            )

            if cc_bufs is not None:
                # ship local K half to the pair partner; overlaps V projection
                ccin_d, ccout_d = cc_bufs
                cci = ccin_d[:]
                cco = ccout_d[:]
                nc.sync.dma_start(cci[0], KT_dst[:])
                if cc_split:
                    pair_allgather(cci[0], cco[0])
                    # KT readback queued right behind gather 1: both halves
                    # land while V/Q still project (uniform across cores)
                    for rep in range(2):
                        nc.sync.dma_start(KT[:, :, rep * SKV:(rep + 1) * SKV],
                                          cco[0, rep])

            # V natural layout: V[si, :] = sum_ki xT[ki, si]^T @ Wv[ki, :]
            proj_run(
                ps_proj,
                [(si, w) for si in range(SKV // P) for w in range(D // 512)],
                0,
                lambda ki, p: xT_sb[:, ki, p[0] * P:(p[0] + 1) * P],
                lambda ki, p: wv_sb[:, ki, p[1] * 512:(p[1] + 1) * 512],
                lambda t, p, tl: cp_eng[t % 2](
                    V_dst[:, p[0], p[1] * 512:(p[1] + 1) * 512], tl[:]),
            )

            if cc_bufs is not None:
                # ship local V half; overlaps Q^T projection
                nrep_blk = SKV // P
                nc.sync.dma_start(cci[1], V_dst[:])
                if cc_split:
                    pair_allgather(cci[1], cco[1])
                    for rep in range(2):
                        nc.sync.dma_start(
                            V[:, rep * nrep_blk:(rep + 1) * nrep_blk, :],
                            cco[1, rep])
                else:
                    # fused: gather whole ccin; output replica-major [rep][t]
                    pair_allgather(ccin_d[:], ccout_d[:])
                    for rep in range(2):
                        nc.sync.dma_start(KT[:, :, rep * SKV:(rep + 1) * SKV],
                                          cco[rep, 0])
                        nc.sync.dma_start(
                            V[:, rep * nrep_blk:(rep + 1) * nrep_blk, :],
                            cco[rep, 1])

            # Q^T (scaled by 1/32): over this core's gathered query columns
            def q_drain(t, p, tl):
                dst = QT[:, p[0], p[1] * 512:(p[1] + 1) * 512]
                if t % 2 == 0:
                    nc.vector.tensor_scalar_mul(dst, tl[:], SCALE)
                else:
                    nc.scalar.mul(dst, tl[:], SCALE)

            proj_run(
                ps_proj,
                [(mi, w) for mi in range(NCH)
                 for w in range(NSLOT * P // 512)],
                0,
                lambda ki, p: wq_sb[:, ki, p[0] * P:(p[0] + 1) * P],
                lambda ki, p: xq_sb[:, ki, p[1] * 512:(p[1] + 1) * 512],
                q_drain,
            )

        ps_big = tc.alloc_tile_pool(name="ps_big", bufs=2 if ps_rebal else 3,
                                    space="PSUM")

    # ---- Phase B/C: attention, software-pipelined over slots ----
    scores_p = tc.alloc_tile_pool(name="scores", bufs=3)
    probs_p = tc.alloc_tile_pool(name="probs", bufs=3)
    pT_p = tc.alloc_tile_pool(name="pT", bufs=2)
    stats = tc.alloc_tile_pool(name="stats", bufs=8)
    out_p = tc.alloc_tile_pool(name="outp", bufs=3)
    ps_tr = tc.alloc_tile_pool(name="ps_tr", bufs=2, space="PSUM")
    ps_o = tc.alloc_tile_pool(name="ps_o", bufs=4 if ps_rebal else 3, space="PSUM")

    def emit_scores(j):
        ncols = (2 * j + 2) * P
        scores = scores_p.tile([P, S], F32, tag="scores")
        probs = probs_p.tile([P, S], BF16, tag="probs")
        c = 0
        w = 0
        while c < ncols:
            wc = min(512, ncols - c)
            ps = ps_big.tile([P, 512], F32)
            for ki in range(NCH):
                nc.tensor.matmul(
                    ps[:, :wc],
                    lhsT=QT[:, ki, j * P:(j + 1) * P],
                    rhs=KT[:, ki, c:c + wc],
                    start=(ki == 0), stop=(ki == NCH - 1),
                )
            cp_eng[w % 2](scores[:, c:c + wc], ps[:, :wc])
            c += wc
            w += 1
        # causal mask on the last two kblocks
        nc.vector.tensor_add(
            scores[:, ncols - 2 * P:ncols], scores[:, ncols - 2 * P:ncols], mask_sb[:]
        )
        negm = stats.tile([P, 1], F32, tag="negm")
        nc.vector.reduce_max(negm[:], scores[:, :ncols], axis=mybir.AxisListType.X, negate=True)
        lsum = stats.tile([P, 1], F32, tag="lsum")
        nc.scalar.activation(
            probs[:, :ncols], scores[:, :ncols],
            mybir.ActivationFunctionType.Exp,
            bias=negm[:], scale=1.0, accum_out=lsum[:],
        )
        rinv = stats.tile([P, 1], F32, tag="rinv")
        nc.vector.reciprocal(rinv[:], lsum[:])
        return probs, rinv

    def emit_pv(j, probs, rinv):
        nk = 2 * j + 2
        pT = pT_p.tile([P, NB * P], BF16, tag="pT")
        # transpose 8 blocks into one PSUM bank (2KB = 1024 bf16), one coarse
        # drain per group — at most 2 groups per slot, no rotation wait
        for g in range(0, nk, 8):
            gw = min(8, nk - g)
            tp = ps_tr.tile([P, 8 * P], BF16, tag="tr")
            for i in range(gw):
                nc.tensor.transpose(tp[:, i * P:(i + 1) * P],
                                    probs[:, (g + i) * P:(g + i + 1) * P],
                                    ident_sb[:])
            cp_eng[(g // 8) % 2](pT[:, g * P:(g + gw) * P], tp[:, :gw * P])
        # o0's accumulation finishes before o1's starts: its normalize+store
        # overlaps o1's matmuls, shortening the end-of-kernel drain.
        o0 = ps_o.tile([P, 512], F32, tag="o")
        o1 = ps_o.tile([P, 512], F32, tag="o")
        outt = out_p.tile([P, D], F32, tag="out")
        out_ap = out_d[:].rearrange("(s p) d -> s p d", p=P)[j, :, :]
        for kb in range(nk):
            nc.tensor.matmul(o0[:], lhsT=pT[:, kb * P:(kb + 1) * P],
                             rhs=V[:, kb, 0:512],
                             start=(kb == 0), stop=(kb == nk - 1))
        nc.vector.tensor_scalar_mul(outt[:, 0:512], o0[:], rinv[:])
        nc.sync.dma_start(out_ap[:, 0:512], outt[:, 0:512])
        for kb in range(nk):
            nc.tensor.matmul(o1[:], lhsT=pT[:, kb * P:(kb + 1) * P],
                             rhs=V[:, kb, 512:1024],
                             start=(kb == 0), stop=(kb == nk - 1))
        nc.vector.tensor_scalar_mul(outt[:, 512:1024], o1[:], rinv[:])
        nc.sync.dma_start(out_ap[:, 512:1024], outt[:, 512:1024])

    # asc: small slots first — they depend only on the first-gathered K/V half,
    # hiding collective latency; desc: big slots first.  Depth-2 software
    # pipeline: pv(j) is emitted after scores(j+2), so the early slots'
    # softmax latency (drain -> mask -> max -> exp) hides behind two scores
    # stages instead of one.
    order = list(range(NSLOT)) if asc else list(range(NSLOT - 1, -1, -1))
    pend = []
    for j in order:
        sp = emit_scores(j)
        pend.append((j, *sp))
        if len(pend) > 1:
            emit_pv(*pend.pop(0))
    for p in pend:
        emit_pv(*p)

    for pool in (ps_o, ps_tr, out_p, stats, pT_p, probs_p, scores_p,
                 ps_big, kv, singles):
        pool.release()


@functools.lru_cache(maxsize=16)
def _build(reps=1, cc=True, cc_split=True, asc=True, ps_rebal=False,
           cc_local=False):
    nc = bacc.Bacc("TRN2", target_bir_lowering=False, debug=False,
                   num_devices=N_CORES)
    skv = S // 2 if cc else S
    xT_d = nc.dram_tensor("xT", [D, skv], BF16, kind="ExternalInput")
    xq_d = nc.dram_tensor("xq", [D, NSLOT * P], BF16, kind="ExternalInput")
    wq_d = nc.dram_tensor("wq", [D, D], BF16, kind="ExternalInput")
    wk_d = nc.dram_tensor("wk", [D, D], BF16, kind="ExternalInput")
    wv_d = nc.dram_tensor("wv", [D, D], BF16, kind="ExternalInput")
    mask_d = nc.dram_tensor("mask", [P, 2 * P], F32, kind="ExternalInput")
    ident_d = nc.dram_tensor("ident", [P, P], BF16, kind="ExternalInput")
    out_d = nc.dram_tensor("out", [NSLOT * P, D], F32, kind="ExternalOutput")
    cc_bufs = None
    if cc:
        ccin_d = nc.dram_tensor("ccin", [2, P, NCH, skv], BF16)
        # [tensor (K/V), replica, p, chunk, col] — each AllGather output contiguous
        ccout_d = nc.dram_tensor("ccout", [2, 2, P, NCH, skv], BF16)
        cc_bufs = (ccin_d, ccout_d)

    with tile.TileContext(nc) as tc:
        for _ in range(reps):
            _emit_kernel(nc, tc, xT_d, xq_d, wq_d, wk_d, wv_d, mask_d, ident_d,
                         out_d, cc_bufs=cc_bufs, cc_split=cc_split, asc=asc,
                         ps_rebal=ps_rebal, cc_local=cc_local)
    nc.compile()
    return nc


def _host_inputs(x, Wq, Wk, Wv, cc=True):
    xT = np.ascontiguousarray(x.transpose(0, 2, 1))  # [B, D, S] fp32
    xT_bf = xT.astype(BF)
    w_bf = {"wq": Wq.astype(BF), "wk": Wk.astype(BF), "wv": Wv.astype(BF)}

    tri = np.where(np.arange(P)[:, None] >= np.arange(P)[None, :], 0.0, NEG).astype(np.float32)
    zeros = np.zeros((P, P), np.float32)
    full = np.full((P, P), NEG, np.float32)
    masks = {
        0: np.concatenate([tri, full], axis=1),   # h=0: diag block then dead block
        1: np.concatenate([zeros, tri], axis=1),  # h=1: visible block then diag block
    }
    ident = np.eye(P, dtype=BF)

    in_maps = []
    for c in range(N_CORES):
        b, h = divmod(c, 2)
        qcols = np.concatenate([np.arange((2 * j + h) * P, (2 * j + h + 1) * P)
                                for j in range(NSLOT)])
        xkv = xT_bf[b][:, h * (S // 2):(h + 1) * (S // 2)] if cc else xT_bf[b]
        in_maps.append({
            "xT": np.ascontiguousarray(xkv),
            "xq": np.ascontiguousarray(xT_bf[b][:, qcols]),
            **w_bf,
            "mask": masks[h],
            "ident": ident,
        })
    return in_maps


def _scatter(results):
    out = np.empty((B, S, D), np.float32)
    for c in range(N_CORES):
        b, h = divmod(c, 2)
        oc = results[c]["out"]
        for j in range(NSLOT):
            g = 2 * j + h
            out[b, g * P:(g + 1) * P, :] = oc[j * P:(j + 1) * P, :]
    return out


def run(x, Wq, Wk, Wv, cc=True, cc_split=True, asc=True, **spmd_kwargs):
    nc = _build(cc=cc, cc_split=cc_split, asc=asc)
    in_maps = _host_inputs(np.asarray(x), np.asarray(Wq), np.asarray(Wk),
                           np.asarray(Wv), cc=cc)
    res = run_bass_kernel_spmd(nc, in_maps, core_ids=list(range(N_CORES)), **spmd_kwargs)
    return _scatter(res.results), res


def _spot_check(out, x, Wq, Wk, Wv, rows=(0, 511, 2047), tol=3e-2):
    # Cheap CPU cross-check of a few query rows per batch: catches transient
    # device/tunnel corruption that stays finite.
    x = np.asarray(x, np.float32)
    for b in range(B):
        k = x[b] @ np.asarray(Wk, np.float32)
        v = x[b] @ np.asarray(Wv, np.float32)
        for r in rows:
            q = x[b, r] @ np.asarray(Wq, np.float32)
            s = (k[:r + 1] @ q) / np.float32(32.0)
            s -= s.max()
            p = np.exp(s)
            p /= p.sum()
            ref = p @ v[:r + 1]
            e = np.linalg.norm(out[b, r] - ref) / np.linalg.norm(ref)
            if not np.isfinite(e) or e > tol:
                return False
    return True


def kernel(x, Wq, Wk, Wv):
    # Fallback chain: pair-collective K/V exchange variants first (fastest),
    # then the self-contained no-collective variant. Each step is a fully
    # independent NEFF; a failure, non-finite output, or failed spot check
    # (transient tunnel corruption) moves to the next attempt.
    last_err = None
    for cfg in (dict(cc=True, cc_split=True, asc=True),
                dict(cc=True, cc_split=True, asc=True),
                dict(cc=True, cc_split=False, asc=False),
                dict(cc=False),
                dict(cc=False)):
        try:
            out, _ = run(x, Wq, Wk, Wv, **cfg)
            if np.isfinite(out).all() and _spot_check(out, x, Wq, Wk, Wv):
                return out
            last_err = RuntimeError(f"bad output with {cfg}")
        except Exception as e:  # noqa: BLE001 — retry next variant
            last_err = e
    raise last_err

